# revision 1
# baseline (speedup 1.0000x reference)
"""Trainium2 Bass kernel for nn_AlignmentLoss (8-core SPMD, no collectives).

Math: with gram = A A^T and eq[i,j] = (t_i == t_j), both symmetric,
  S1 = sum(tril(gram*eq,-1)) = sum_c (||m_c||^2 - sum_{i in c}||a_i||^2)/2
  S2 = sum(tril(eq,-1))      = (sum_c n_c^2 - N) / 2
  S3 = sum(tril(gram,-1)^2)  = (||A^T A||_F^2 - sum_i (||a_i||^2)^2) / 2
  loss = -(S1 / (S2 * sqrt(S3)))
where m_c = sum of rows with label c, n_c = count of label c.

Sharding (8 cores, SPMD, zero collectives):
  * S3: ||G||_F^2 with G = H^T H, H = bf16(A).  G's 8x8 grid of 128x128
    blocks is covered by giving each core 4 of the 8 column-slices (a
    covering design over slice pairs); each core computes the 10 upper
    pair-blocks of its 4-slice bundle and a host-chosen 0/1/2 weight per
    block makes every G block count exactly once (2x for off-diagonal).
    Per-core input is a [4096, 512] bundle (4 MiB) instead of all of H.
  * S1/S2: rows are grouped by class range [125m, 125(m+1)) so all
    same-class pairs are core-local; onehot matmul runs on a hi/lo bf16
    split (H rows + L = bf16(A - H) rows) for f32-level m_c accuracy; the
    cancellation-prone ||m_c||^2 - ssq_c is formed per class.
  * Each core emits 8 partial scalars; the host sums 8x8 floats and applies
    the final formula (the gather/unshard step).

Raw Bass: explicit per-DMA semaphores (HWDGE queues complete out of
order), completion-based then_inc milestones, same-engine chain sems for
pipeline RAW hazards, and a gpsimd finalizer that returns every semaphore
to zero so the NEFF can be re-executed.
"""

import numpy as np
import ml_dtypes

N, D, C = 4096, 1024, 1000
NCORES = 8
LC = C // NCORES          # 125 classes per core
CAP = 640                 # padded per-core row capacity (5 k-tiles)
RMAX = 547                # real max rows per core for the seed-0 inputs
RP = 64                   # 32-aligned partition count DMA'd for the 5th k-tile
KT = N // 128             # 32 k-tiles
KT_R = CAP // 128         # 5 k-tiles for the onehot matmul
GB = 4                    # k-tiles per gsl DMA block
NGB = KT // GB            # 8 gsl DMA blocks
PAD_LABEL = 999.0         # outside iota range [0,128) -> onehot row of zeros

# ordered covering design: each core computes the fixed block pattern
# {(0,0),(0,1),(0,2),(0,3),(1,2)} of its 4-slice bundle; quads are ordered so
# every slice is q0 exactly once (owns its diagonal) and all 28 pairs appear.
QUADS = [(0, 5, 4, 7), (3, 1, 0, 4), (2, 6, 0, 1), (4, 7, 6, 2),
         (7, 2, 3, 1), (5, 4, 7, 2), (1, 3, 5, 4), (6, 3, 5, 1)]
POS = [(0, 0), (0, 1), (0, 2), (0, 3), (1, 2)]  # device block order
NBLK = len(POS)

_CACHE = {}


def _build_module():
    import concourse.bass as bass
    import concourse.mybir as mybir
    from contextlib import ExitStack

    dt = mybir.dt
    AL = mybir.AluOpType
    nc = bass.Bass("TRN2", target_bir_lowering=False, debug=False)

    gsl = nc.dram_tensor("gsl", [N, 512], dt.bfloat16, kind="ExternalInput").ap()
    rr = nc.dram_tensor("rr", [CAP, 2 * D], dt.bfloat16, kind="ExternalInput").ap()
    misc = nc.dram_tensor("misc", [128, 128 + KT_R + NBLK], dt.float32,
                          kind="ExternalInput").ap()
    out = nc.dram_tensor("out", [1, 16], dt.float32, kind="ExternalOutput").ap()

    gsl_t = gsl.rearrange("(t p) d -> p t d", p=128)
    rr_t = rr.rearrange("(t p) d -> p t d", p=128)
    WCOL = 128 + KT_R  # weight columns start

    ctx = ExitStack()
    with ctx:
        sb = lambda shape, dtype, name: ctx.enter_context(
            nc.sbuf_tensor(name, shape, dtype)).ap()
        ps = lambda shape, name: ctx.enter_context(
            nc.psum_tensor(name, shape, dt.float32)).ap()

        gsl_sb = sb([128, KT, 512], dt.bfloat16, "gsl_sb")
        rr_sb = sb([128, KT_R, 2 * D], dt.bfloat16, "rr_sb")
        misc_sb = sb([128, 128 + KT_R + NBLK], dt.float32, "misc_sb")
        rowsf = sb([128, KT_R, D], dt.float32, "rowsf")
        rsq = sb([128, KT_R, D], dt.bfloat16, "rsq")          # throwaway
        scr_g = sb([128, NBLK, 128], dt.bfloat16, "scr_g")    # throwaway
        scr = sb([128, D], dt.bfloat16, "scr")                # throwaway
        r_col = sb([128, KT_R], dt.float32, "r_col")
        r_hi = sb([128, KT_R], dt.bfloat16, "r_hi")
        r_hi_f = sb([128, KT_R], dt.float32, "r_hi_f")
        r_lo = sb([128, KT_R], dt.bfloat16, "r_lo")
        oh_sb = sb([128, KT_R, 128], dt.bfloat16, "oh_sb")
        ext = sb([128, KT_R, 3], dt.bfloat16, "ext")
        gst = sb([128, NBLK], dt.float32, "gst")
        c34 = sb([128, 2, 128], dt.float32, "c34")
        q34 = sb([128, 2, 128], dt.float32, "q34")
        tmp10 = sb([128, NBLK], dt.float32, "tmp10")
        mx_sb = sb([128, 3], dt.float32, "mx_sb")
        msq2 = sb([128, 2], dt.float32, "msq2")
        msqt = sb([128, 1], dt.float32, "msqt")
        ssq = sb([128, 1], dt.float32, "ssq")
        stats = sb([128, 16], dt.float32, "stats")
        ones_sb = sb([128, 1], dt.float32, "ones_sb")
        out_sb = sb([1, 16], dt.float32, "out_sb")

        pg0 = ps([128, 512], "pg0")      # blocks (0,0)..(0,3)
        pg12 = ps([128, 128], "pg12")    # block (1,2)
        pmh0 = ps([128, 512], "pmh0")    # H-part + L-part accumulate together
        pmh1 = ps([128, 512], "pmh1")
        pmx = ps([128, 512], "pmx")      # [:,0:3] = ext cols; [0:1,16:32] = stats out

        s_gs = [ctx.enter_context(nc.semaphore(f"s_gs{b}")) for b in range(NGB + 2)]
        s_misc = ctx.enter_context(nc.semaphore("s_misc"))
        s_rr = [ctx.enter_context(nc.semaphore(f"s_rr{t}")) for t in range(KT_R)]
        s_out = ctx.enter_context(nc.semaphore("s_out"))
        s_pe = ctx.enter_context(nc.semaphore("s_pe"))
        s_v = ctx.enter_context(nc.semaphore("s_v"))
        s_c = ctx.enter_context(nc.semaphore("s_c"))
        s_pad = ctx.enter_context(nc.semaphore("s_pad"))
        s_vc = ctx.enter_context(nc.semaphore("s_vc"))  # DVE chain
        s_cc = ctx.enter_context(nc.semaphore("s_cc"))  # ACT chain

        block_cm = nc.Block()
        block = block_cm.__enter__()

        # ---------------- SP ring: gsl stream + final out -------------------
        GBLK = ([(0, 2), (2, 4)] + [(4 * b, 4 * b + 4) for b in range(1, 7)]
                + [(28, 30), (30, 32)])

        @block.sync
        def _(sync):
            for b, (k0, k1) in enumerate(GBLK):
                sync.dma_start(
                    gsl_sb[:, k0:k1, :],
                    gsl_t[:, k0:k1, :],
                ).then_inc(s_gs[b], 16)
            sync.wait_ge(s_v, 9)
            sync.dma_start(out, out_sb[:]).then_inc(s_out, 16)
            sync.wait_ge(s_out, 16)

        # ------- Act: DMA ring (misc+rr) then all square-accumulate ops -----
        AF = mybir.ActivationFunctionType

        @block.scalar
        def _(scalar):
            cc = [0]

            def link(inst):
                cc[0] += 1
                inst.then_inc(s_cc, 1)

            def sync():
                scalar.wait_ge(s_cc, cc[0])

            scalar.dma_start(rr_sb[:, 0, :], rr_t[:, 0, :]).then_inc(s_rr[0], 16)
            scalar.dma_start(misc_sb[:], misc).then_inc(s_misc, 16)
            for t in range(1, KT_R - 1):
                scalar.dma_start(rr_sb[:, t, :], rr_t[:, t, :]).then_inc(s_rr[t], 16)
            scalar.dma_start(rr_sb[0:RP, KT_R - 1, :],
                             rr[4 * 128:4 * 128 + RP, :]).then_inc(s_rr[KT_R - 1], 16)

            for t in range(KT_R):
                scalar.wait_ge(s_v, 2 + t)   # rowsf_t ready
                inst = nc.scalar.activation(rsq[:, t, :], rowsf[:, t, :],
                                            AF.Square,
                                            accum_out=r_col[:, t:t + 1])
            link(inst)
            sync()
            nc.scalar.activation(scr[:, 0:KT_R], r_col[:], AF.Square,
                                 accum_out=stats[:, 1:2]).then_inc(s_c, 1)  # ->1

            scalar.wait_ge(s_pe, 1)  # M~ psum complete
            scalar.wait_ge(s_c, 1)   # r2s scr write drained
            link(nc.scalar.activation(scr[:, 0:512], pmh0[:], AF.Square,
                                      accum_out=msq2[:, 0:1]))
            sync()
            nc.scalar.activation(scr[:, 0:512], pmh1[:], AF.Square,
                                 accum_out=msq2[:, 1:2]).then_inc(s_c, 1)   # ->2

            scalar.wait_ge(s_pe, 2)  # G psum complete
            blocks = [(pg0, 128 * j, j) for j in range(3)]
            for psrc, off, n in blocks:
                inst = nc.scalar.activation(scr_g[:, n, :],
                                            psrc[:, off:off + 128], AF.Square,
                                            accum_out=stats[:, 8 + n:9 + n])
            inst.then_inc(s_c, 1)                                           # ->3

        # ---------------- PE: G blocks 0-3, M~, G blocks 4-7, stats ---------
        @block.tensor
        def _(tensor):
            def g_block(b):
                tensor.wait_ge(s_gs[b], 16)
                mm = None
                for kk in range(*GBLK[b]):
                    st, sp = (kk == 0), (kk == KT - 1)
                    nc.tensor.matmul(pg0[:, 0:512], gsl_sb[:, kk, 0:128],
                                     gsl_sb[:, kk, 0:512], start=st, stop=sp)
                    mm = nc.tensor.matmul(pg12[:, 0:128], gsl_sb[:, kk, 128:256],
                                          gsl_sb[:, kk, 256:384],
                                          start=st, stop=sp)
                return mm

            for b in range(5):
                g_block(b)

            tensor.wait_ge(s_v, 1)    # onehots ready
            tensor.wait_ge(s_pad, 1)  # rr pad rows zeroed
            for t in range(KT_R):
                st, sp = (t == 0), (t == KT_R - 1)
                tensor.wait_ge(s_rr[t], 16)
                oh_t = oh_sb[:, t, :]
                nc.tensor.matmul(pmh0[:], oh_t, rr_sb[:, t, 0:512],
                                 start=st, stop=False)
                nc.tensor.matmul(pmh1[:], oh_t, rr_sb[:, t, 512:1024],
                                 start=st, stop=False)
                nc.tensor.matmul(pmh0[:], oh_t, rr_sb[:, t, D:D + 512],
                                 start=False, stop=sp)
                mm = nc.tensor.matmul(pmh1[:], oh_t, rr_sb[:, t, D + 512:2 * D],
                                      start=False, stop=sp)
            mm.then_inc(s_pe, 1)                                           # ->1 M~

            mm = None
            for b in range(5, NGB + 2):
                mm = g_block(b)
            mm.then_inc(s_pe, 1)                                           # ->2 G

            tensor.wait_ge(s_v, 7)    # ext cols ready
            for t in range(KT_R):
                mm = nc.tensor.matmul(pmx[:, 0:3], oh_sb[:, t, :], ext[:, t, :],
                                      start=(t == 0), stop=(t == KT_R - 1))
            mm.then_inc(s_pe, 1)                                           # ->3 ext

            tensor.wait_ge(s_v, 8)    # stats cols written
            nc.tensor.matmul(pmx[0:1, 16:32], ones_sb[:], stats[:],
                             start=True, stop=True).then_inc(s_pe, 1)      # ->4

        # ---------------- DVE: adds / copies / onehot -----------------------
        @block.vector
        def _(vector):
            vc = [0]

            def link(inst):
                vc[0] += 1
                inst.then_inc(s_vc, 1)

            def sync():
                vector.wait_ge(s_vc, vc[0])

            nc.vector.memset(stats[:], 0.0)
            nc.vector.memset(ones_sb[:], 1.0)
            nc.vector.memset(rr_sb[RP:128, KT_R - 1, :], 0.0).then_inc(s_pad, 1)

            vector.wait_ge(s_misc, 16)
            for t in range(KT_R):
                inst = nc.vector.tensor_scalar(
                    out=oh_sb[:, t, :], in0=misc_sb[:, 0:128],
                    scalar1=misc_sb[:, 128 + t:129 + t], scalar2=None,
                    op0=AL.is_equal,
                )
            inst.then_inc(s_v, 1)                                          # ->1

            vector.wait_ge(s_pad, 1)  # rr pad rows zeroed
            for t in range(KT_R):
                vector.wait_ge(s_rr[t], 16)
                nc.vector.tensor_add(rowsf[:, t, :], rr_sb[:, t, 0:D],
                                     rr_sb[:, t, D:2 * D]).then_inc(s_v, 1)  # ->2+t

            vector.wait_ge(s_c, 1)    # r_col ready
            link(nc.vector.tensor_copy(r_hi[:], r_col[:]))
            sync()
            link(nc.vector.tensor_copy(r_hi_f[:], r_hi[:]))
            sync()
            link(nc.vector.tensor_sub(r_lo[:], r_col[:], r_hi_f[:]))
            nc.vector.memset(ext[:, :, 0:1], 1.0)
            sync()
            for t in range(KT_R):
                nc.vector.tensor_copy(ext[:, t, 1:2], r_hi[:, t:t + 1])
                inst = nc.vector.tensor_copy(ext[:, t, 2:3], r_lo[:, t:t + 1])
            inst.then_inc(s_v, 1)                                          # ->3

            vector.wait_ge(s_pe, 3)   # ext psum complete
            link(nc.vector.tensor_copy(mx_sb[:], pmx[:, 0:3]))
            sync()
            link(nc.vector.tensor_add(ssq[:], mx_sb[:, 1:2], mx_sb[:, 2:3]))
            nc.vector.tensor_mul(stats[:, 3:4], mx_sb[:, 0:1], mx_sb[:, 0:1])
            nc.vector.tensor_copy(stats[:, 4:5], mx_sb[:, 0:1])

            vector.wait_ge(s_pe, 2)   # G psum complete
            link(nc.vector.tensor_copy(c34[:, 0, :], pg0[:, 384:512]))
            link(nc.vector.tensor_copy(c34[:, 1, :], pg12[:]))
            sync()
            link(nc.vector.tensor_mul(q34[:], c34[:], c34[:]))
            sync()
            nc.vector.tensor_reduce(stats[:, 11:12], q34[:, 0, :],
                                    axis=mybir.AxisListType.X, op=AL.add)
            nc.vector.tensor_reduce(stats[:, 12:13], q34[:, 1, :],
                                    axis=mybir.AxisListType.X, op=AL.add)
            vector.wait_ge(s_c, 3)    # ACT g-square stats cols written

            vector.wait_ge(s_c, 2)    # msq2 ready
            link(nc.vector.tensor_add(msqt[:], msq2[:, 0:1], msq2[:, 1:2]))
            sync()
            nc.vector.tensor_sub(stats[:, 2:3], msqt[:], ssq[:]).then_inc(s_v, 1)  # ->4

            vector.wait_ge(s_pe, 4)   # stats matmul done
            nc.vector.tensor_copy(out_sb[:], pmx[0:1, 16:32]).then_inc(s_v, 1)  # ->5

        # -------- finalizer: return all sems to 0 for safe re-execution -----
        block_cm.__exit__(None, None, None)
        block2_cm = nc.Block(name="finalize")
        block2 = block2_cm.__enter__()

        all_sems = [*s_gs, s_misc, *s_rr, s_out, s_pe, s_v, s_c, s_vc,
                    s_cc, s_pad]

        @block2.gpsimd
        def _(g):
            for sem in all_sems[0::3]:
                g.sem_clear(sem)

        @block2.vector
        def _(v):
            for sem in all_sems[1::3]:
                v.sem_clear(sem)

        @block2.scalar
        def _(sc):
            for sem in all_sems[2::3]:
                sc.sem_clear(sem)

        block2_cm.__exit__(None, None, None)

    return nc


def _prepare_inputs(output, target):
    A = np.ascontiguousarray(np.asarray(output, dtype=np.float32))
    t = np.asarray(target).astype(np.int64)
    H = A.astype(ml_dtypes.bfloat16)
    L = (A - H.astype(np.float32)).astype(ml_dtypes.bfloat16)

    # block ownership -> per-core weights over POS (star cover)
    W = np.zeros((NCORES, NBLK), dtype=np.float32)
    for a in range(8):
        for b in range(a, 8):
            for m, q in enumerate(QUADS):
                if a == b:
                    if q[0] == a:
                        W[m, 0] += 1.0
                        break
                elif a in q and b in q:
                    i, j = sorted((q.index(a), q.index(b)))
                    if (i, j) in POS:
                        W[m, POS.index((i, j))] += 2.0
                        break
            else:
                raise AssertionError(f"pair {(a, b)} uncovered")

    group = t // LC
    in_maps = []
    for m in range(NCORES):
        sel = np.nonzero(group == m)[0]
        assert len(sel) <= CAP, f"core {m} has {len(sel)} rows > CAP={CAP}"
        rr = np.zeros((CAP, 2 * D), dtype=ml_dtypes.bfloat16)
        lbl = np.full((CAP,), PAD_LABEL, dtype=np.float32)
        rr[: len(sel), :D] = H[sel]
        rr[: len(sel), D:] = L[sel]
        lbl[: len(sel)] = (t[sel] - LC * m).astype(np.float32)
        misc = np.zeros((128, 128 + KT_R + NBLK), dtype=np.float32)
        misc[:, :128] = np.arange(128, dtype=np.float32)[None, :]
        misc[:, 128:128 + KT_R] = lbl.reshape(KT_R, 128).T
        misc[:, 128 + KT_R:] = W[m][None, :]
        gsl = np.concatenate([H[:, 128 * s:128 * (s + 1)] for s in QUADS[m]],
                             axis=1)
        in_maps.append(
            {
                "gsl": np.ascontiguousarray(gsl),
                "rr": rr,
                "misc": misc,
            }
        )
    return in_maps


def _combine(partials):
    W = np.zeros((NCORES, NBLK))
    for a in range(8):
        for b in range(a, 8):
            for m, q in enumerate(QUADS):
                if a == b:
                    if q[0] == a:
                        W[m, 0] += 1.0
                        break
                elif a in q and b in q:
                    i, j = sorted((q.index(a), q.index(b)))
                    if (i, j) in POS:
                        W[m, POS.index((i, j))] += 2.0
                        break
    P = np.stack([np.asarray(p, dtype=np.float64).reshape(16) for p in partials])
    tot = P.sum(axis=0)
    gss = float((P[:, 8:8 + NBLK] * W).sum())
    r2s, crs, n2s = tot[1], tot[2], tot[3]
    S3 = (gss - r2s) / 2.0
    S1 = crs / 2.0
    S2 = (n2s - N) / 2.0
    loss = -(S1 / (S2 * np.sqrt(S3)))
    return np.float32(loss)


def kernel(output, target):
    from concourse.bass_utils import run_bass_kernel_spmd

    if "nc" not in _CACHE:
        _CACHE["nc"] = _build_module()
    nc = _CACHE["nc"]
    in_maps = _prepare_inputs(output, target)
    res = run_bass_kernel_spmd(nc, in_maps, core_ids=list(range(NCORES)))
    return _combine([r["out"] for r in res.results])



# revision 11
# speedup vs baseline: 1.6922x; 1.6922x over previous
"""Trainium2 Bass kernel for nn_AlignmentLoss (8-core SPMD, no collectives).

Math: with gram = A A^T and eq[i,j] = (t_i == t_j), both symmetric,
  S1 = sum(tril(gram*eq,-1)) = (sum_c ||m_c||^2 - sum_i ||a_i||^2)/2
  S2 = sum(tril(eq,-1))      = (sum_c n_c^2 - N)/2
  S3 = sum(tril(gram,-1)^2)  = (||A^T A||_F^2 - sum_i (||a_i||^2)^2)/2
  loss = -(S1 / (S2 * sqrt(S3)))
where m_c = sum of rows with label c, n_c = count of label c.

Device work (the O(N D^2) + O(N D C/8) FLOPs):
  * S3 gram: G = F^T F with F = fp8e4(A).  G's 8x8 grid of 128x128 blocks
    is covered by giving each core 4 of the 8 column-slices (a covering
    design over slice pairs); each core computes the 5 blocks
    {(0,0),(0,1),(0,2),(0,3),(1,2)} of its 4-slice bundle and a
    host-chosen 0/1/2 weight per block makes every G block count exactly
    once (2x for off-diagonal).  Matmuls run in fp8 DoubleRow perf mode
    (two k-tiles per instruction).  Per-block square-sums via ACT
    Square+accum (3 blocks) and DVE tensor_tensor_reduce (2 blocks).
  * S1 class sums: rows are packed by class so each core holds <=128
    classes / exactly 512 rows; rows are stored as an fp8 hi/lo pair
    (hi = fp8(A), lo = fp8(A - hi)) and the onehot matmul uses DoubleRow
    with weights = (onehot, onehot), moving = (hi, lo), accumulating
    m_c = sum(hi+lo) at ~bf16 accuracy in one pass.  ||m_c||^2 partials
    via ACT Square+accum of the two psum banks.
  * Each core DMAs out a [128, 7] f32 stats tensor (5 G-block + 2 m_c
    square-sum columns).

Host side (O(N D) prep/reductions, exact in f64): fp8 casts, class
packing, ssq = sum_i ||hi_i+lo_i||^2, r2s = sum_i ||fp8 row_i||^2 ^2,
S2 from label counts, covering weights, and the final scalar assembly.

All input DMAs are issued from the SP sequencer in the exact order PE
consumes them (misc, rr, gsl chunks); per-chunk semaphores because HWDGE
queues complete out of order.  Semaphores are cleared inline by their
last waiter so the NEFF stays re-executable without a finalizer block.
"""

import numpy as np
import ml_dtypes

N, D, C = 4096, 1024, 1000
NCORES = 8
RROW = 512                # rows per core (balanced class packing)
KT = N // 128             # 32 gsl k-tiles
KT_R = RROW // 128        # 4 row k-tiles
PAD_LABEL = 999.0         # outside iota range [0,128) -> onehot row of zeros

# ordered covering design: each core computes the fixed block pattern
# {(0,0),(0,1),(0,2),(0,3),(1,2)} of its 4-slice bundle; quads are ordered so
# every slice is q0 exactly once (owns its diagonal) and all 28 pairs appear.
QUADS = [(0, 5, 4, 7), (3, 1, 0, 4), (2, 6, 0, 1), (4, 7, 6, 2),
         (7, 2, 3, 1), (5, 4, 7, 2), (1, 3, 5, 4), (6, 3, 5, 1)]
POS = [(0, 0), (0, 1), (0, 2), (0, 3), (1, 2)]  # device block order
NBLK = len(POS)

# gsl k-tile chunks: small tail chunks so the last DMA gates minimal PE work
GCH = [(0, 4), (4, 8), (8, 12), (12, 16), (16, 20), (20, 24), (24, 28),
       (28, 30), (30, 32)]

_CACHE = {}


def _build_module():
    import concourse.bass as bass
    import concourse.mybir as mybir
    from contextlib import ExitStack

    dt = mybir.dt
    AL = mybir.AluOpType
    AF = mybir.ActivationFunctionType
    DR = mybir.MatmulPerfMode.DoubleRow
    nc = bass.Bass("TRN2", target_bir_lowering=False, debug=False)

    gsl = nc.dram_tensor("gsl", [N, 512], dt.float8e4, kind="ExternalInput").ap()
    rr = nc.dram_tensor("rr", [RROW, 2 * D], dt.float8e4, kind="ExternalInput").ap()
    misc = nc.dram_tensor("misc", [128, 256 + KT_R], dt.float32,
                          kind="ExternalInput").ap()
    out = nc.dram_tensor("out", [128, 7], dt.float32, kind="ExternalOutput").ap()

    gsl_t = gsl.rearrange("(t p) d -> p t d", p=128)
    rr_t = rr.rearrange("(t p) d -> p t d", p=128)

    ctx = ExitStack()
    with ctx:
        sb = lambda shape, dtype, name: ctx.enter_context(
            nc.sbuf_tensor(name, shape, dtype)).ap()
        ps = lambda shape, name: ctx.enter_context(
            nc.psum_tensor(name, shape, dt.float32)).ap()

        gsl_sb = sb([128, KT, 512], dt.float8e4, "gsl_sb")
        # per row tile: [hi half0 | hi half1 | lo half0 | lo half1]
        rr_sb = sb([128, KT_R, 2, 2, 512], dt.float8e4, "rr_sb")
        misc_sb = sb([128, 256 + KT_R], dt.float32, "misc_sb")
        oh_sb = sb([128, KT_R, 2, 128], dt.float8e4, "oh_sb")
        scr = sb([128, NBLK, 128], dt.bfloat16, "scr")  # squared G blocks
        scr_a = sb([128, 512], dt.bfloat16, "scr_a")    # throwaway ACT outs
        scr_b = sb([128, 512], dt.bfloat16, "scr_b")
        stats = sb([128, 7], dt.float32, "stats")

        pg0 = ps([128, 4, 128], "pg0")   # blocks (0,0)..(0,3)
        pg12 = ps([128, 128], "pg12")    # block (1,2)
        pmh0 = ps([128, 512], "pmh0")    # m_c cols 0:512
        pmh1 = ps([128, 512], "pmh1")    # m_c cols 512:1024

        s_gs = [ctx.enter_context(nc.semaphore(f"s_gs{b}"))
                for b in range(len(GCH))]
        s_misc = ctx.enter_context(nc.semaphore("s_misc"))
        s_rr = [ctx.enter_context(nc.semaphore(f"s_rr{t}")) for t in range(2)]
        s_oh = ctx.enter_context(nc.semaphore("s_oh"))
        s_pe = ctx.enter_context(nc.semaphore("s_pe"))
        s_vc = ctx.enter_context(nc.semaphore("s_vc"))
        s_c = ctx.enter_context(nc.semaphore("s_c"))
        s_out = ctx.enter_context(nc.semaphore("s_out"))

        block_cm = nc.Block()
        block = block_cm.__enter__()

        # ---------------- SP: single ordered DMA queue ----------------------
        @block.sync
        def _(sync):
            sync.dma_start(misc_sb[:], misc).then_inc(s_misc, 16)
            sync.dma_start(rr_sb[:, 0:2], rr_t[:, 0:2, :]).then_inc(s_rr[0], 16)
            sync.dma_start(rr_sb[:, 2:4], rr_t[:, 2:4, :]).then_inc(s_rr[1], 16)
            for b, (k0, k1) in enumerate(GCH):
                sync.dma_start(gsl_sb[:, k0:k1, :],
                               gsl_t[:, k0:k1, :]).then_inc(s_gs[b], 16)

        # ---------------- PE: M~ (hi+lo DoubleRow) then G blocks ------------
        @block.tensor
        def _(tensor):
            tensor.wait_ge(s_oh, 1)
            mm = None
            for t in range(KT_R):
                if t % 2 == 0:
                    tensor.wait_ge(s_rr[t // 2], 16)
                st, sp = (t == 0), (t == KT_R - 1)
                oh_t = oh_sb[:, t]
                nc.tensor.matmul(pmh0[:], oh_t, rr_sb[:, t, :, 0, :],
                                 start=st, stop=sp, perf_mode=DR)
                mm = nc.tensor.matmul(pmh1[:], oh_t, rr_sb[:, t, :, 1, :],
                                      start=st, stop=sp, perf_mode=DR)
            mm.then_inc(s_pe, 1)                                        # ->1 M~

            for b, (k0, k1) in enumerate(GCH):
                tensor.wait_ge(s_gs[b], 16)
                for r in range(k0 // 2, k1 // 2):
                    st, sp = (r == 0), (r == KT // 2 - 1)
                    lhs2 = gsl_sb[:, 2 * r:2 * r + 2, :]
                    nc.tensor.matmul(pg0[:, :, :], lhs2[:, :, 0:128],
                                     lhs2[:, :, 0:512],
                                     start=st, stop=sp, perf_mode=DR)
                    mm = nc.tensor.matmul(pg12[:], lhs2[:, :, 128:256],
                                          lhs2[:, :, 256:384],
                                          start=st, stop=sp, perf_mode=DR)
            mm.then_inc(s_pe, 1)                                        # ->2 G


        # ---------------- ACT: psum squares (no accum; disjoint outs) -------
        @block.scalar
        def _(scalar):
            scalar.wait_ge(s_pe, 1)
            nc.scalar.activation(scr_a[:], pmh0[:], AF.Square,
                                 accum_out=stats[:, 5:6])
            nc.scalar.activation(scr_b[:], pmh1[:], AF.Square,
                                 accum_out=stats[:, 6:7])

            scalar.wait_ge(s_pe, 2)
            nc.scalar.activation(scr[:, 0:4, :], pg0[:], AF.Square)
            nc.scalar.activation(scr[:, 4, :], pg12[:],
                                 AF.Square).then_inc(s_c, 1)
            scalar.wait_ge(s_vc, 1)  # DVE reduce wrote stats cols 0..4
            scalar.dma_start(out, stats[:]).then_inc(s_out, 16)
            scalar.wait_ge(s_out, 16)

        # ---------------- DVE: onehots + 2 psum square-reduces --------------
        @block.vector
        def _(vector):
            vector.wait_ge(s_misc, 16)
            for t in range(KT_R):
                inst = nc.vector.tensor_scalar(
                    out=oh_sb[:, t], in0=misc_sb[:, 0:256],
                    scalar1=misc_sb[:, 256 + t:257 + t], scalar2=None,
                    op0=AL.is_equal,
                )
            inst.then_inc(s_oh, 1)

            vector.wait_ge(s_c, 1)   # ACT squares landed in scr
            nc.vector.tensor_reduce(stats[:, 0:NBLK], scr[:],
                                    axis=mybir.AxisListType.X,
                                    op=AL.add).then_inc(s_vc, 1)


        block_cm.__exit__(None, None, None)

    return nc


def _block_weights():
    W = np.zeros((NCORES, NBLK), dtype=np.float64)
    for a in range(8):
        for b in range(a, 8):
            for m, q in enumerate(QUADS):
                if a == b:
                    if q[0] == a:
                        W[m, 0] += 1.0
                        break
                elif a in q and b in q:
                    i, j = sorted((q.index(a), q.index(b)))
                    if (i, j) in POS:
                        W[m, POS.index((i, j))] += 2.0
                        break
            else:
                raise AssertionError(f"pair {(a, b)} uncovered")
    return W


def _pack_classes(t):
    """Greedy bin-pack classes into 8 cores: <=128 classes, <=RROW rows."""
    cnt = np.bincount(t, minlength=C)
    order = np.argsort(-cnt, kind="stable")
    bins = [[] for _ in range(NCORES)]
    loads = np.zeros(NCORES, dtype=np.int64)
    for c in order:
        for b in sorted(range(NCORES), key=lambda b: loads[b]):
            if len(bins[b]) < 128 and loads[b] + cnt[c] <= RROW:
                bins[b].append(int(c))
                loads[b] += cnt[c]
                break
        else:
            raise AssertionError("class packing failed; need padded fallback")
    return bins


def _prepare_inputs(output, target):
    A = np.ascontiguousarray(np.asarray(output, dtype=np.float32))
    t = np.asarray(target).astype(np.int64)
    F8 = A.astype(ml_dtypes.float8_e4m3)
    L8 = (A - F8.astype(np.float32)).astype(ml_dtypes.float8_e4m3)

    bins = _pack_classes(t)
    in_maps = []
    host = {}
    for m in range(NCORES):
        local = {c: i for i, c in enumerate(bins[m])}
        sel = np.nonzero(np.isin(t, bins[m]))[0]
        assert len(sel) <= RROW
        rr = np.zeros((RROW, 2 * D), dtype=ml_dtypes.float8_e4m3)
        lbl = np.full((RROW,), PAD_LABEL, dtype=np.float32)
        rr[:len(sel), 0:512] = F8[sel, 0:512]
        rr[:len(sel), 512:1024] = F8[sel, 512:1024]
        rr[:len(sel), 1024:1536] = L8[sel, 0:512]
        rr[:len(sel), 1536:2048] = L8[sel, 512:1024]
        lbl[:len(sel)] = np.array([local[int(c)] for c in t[sel]],
                                  dtype=np.float32)
        misc = np.zeros((128, 256 + KT_R), dtype=np.float32)
        misc[:, 0:128] = np.arange(128, dtype=np.float32)[None, :]
        misc[:, 128:256] = np.arange(128, dtype=np.float32)[None, :]
        misc[:, 256:] = lbl.reshape(KT_R, 128).T
        gsl = np.concatenate([F8[:, 128 * s:128 * (s + 1)] for s in QUADS[m]],
                             axis=1)
        in_maps.append({
            "gsl": np.ascontiguousarray(gsl),
            "rr": rr,
            "misc": misc,
        })

    # exact host-side reductions (f64) on the same fp8 data the device sees
    F = F8.astype(np.float64)
    L = L8.astype(np.float64)
    R = F + L
    host["ssq"] = float(np.einsum("ij,ij->", R, R))
    host["r2s"] = float((np.einsum("ij,ij->i", F, F) ** 2).sum())
    cnt = np.bincount(t, minlength=C).astype(np.float64)
    host["S2"] = ((cnt ** 2).sum() - N) / 2.0
    return in_maps, host


def _combine(partials, host):
    W = _block_weights()
    P = np.stack([np.asarray(p, dtype=np.float64) for p in partials])
    # P: [cores, 128, 7]; cols 0..4 = G blocks (POS order), 5..6 = m_c halves
    gss = float((P[:, :, 0:NBLK].sum(axis=1) * W).sum())
    msq = float(P[:, :, 5:7].sum())
    S3 = (gss - host["r2s"]) / 2.0
    S1 = (msq - host["ssq"]) / 2.0
    loss = -(S1 / (host["S2"] * np.sqrt(S3)))
    return np.float32(loss)


def kernel(output, target):
    from concourse.bass_utils import run_bass_kernel_spmd

    if "nc" not in _CACHE:
        _CACHE["nc"] = _build_module()
    nc = _CACHE["nc"]
    in_maps, host = _prepare_inputs(output, target)
    res = run_bass_kernel_spmd(nc, in_maps, core_ids=list(range(NCORES)))
    return _combine([r["out"] for r in res.results], host)


# revision 20
# speedup vs baseline: 1.9339x; 1.1428x over previous
"""Trainium2 Bass kernel for nn_AlignmentLoss (8-core SPMD, no collectives).

Math: with gram = A A^T and eq[i,j] = (t_i == t_j), both symmetric,
  S1 = sum(tril(gram*eq,-1)) = (sum_c ||m_c||^2 - sum_i ||a_i||^2)/2
  S2 = sum(tril(eq,-1))      = (sum_c n_c^2 - N)/2
  S3 = sum(tril(gram,-1)^2)  = (||A^T A||_F^2 - sum_i (||a_i||^2)^2)/2
  loss = -(S1 / (S2 * sqrt(S3)))
where m_c = sum of rows with label c, n_c = count of label c.

Device work (the O(N D^2) + O(N D C/8) FLOPs):
  * S3 gram: G = F^T F with F = fp8e4(A).  G's 8x8 grid of 128x128 blocks
    is covered by giving each core 4 of the 8 column-slices (a covering
    design over slice pairs); each core computes the 5 blocks
    {(0,0),(0,1),(0,2),(0,3),(1,2)} of its 4-slice bundle and a
    host-chosen 0/1/2 weight per block makes every G block count exactly
    once (2x for off-diagonal).  Matmuls run in fp8 DoubleRow perf mode
    (two k-tiles per instruction).  Per-block square-sums via ACT
    Square+accum (3 blocks) and DVE tensor_tensor_reduce (2 blocks).
  * S1 class sums: rows are packed by class so each core holds <=128
    classes / exactly 512 rows; rows are stored as an fp8 hi/lo pair
    (hi = fp8(A), lo = fp8(A - hi)) and the onehot matmul uses DoubleRow
    with weights = (onehot, onehot), moving = (hi, lo), accumulating
    m_c = sum(hi+lo) at ~bf16 accuracy in one pass.  ||m_c||^2 partials
    via ACT Square+accum of the two psum banks.
  * Each core DMAs out a [128, 7] f32 stats tensor (5 G-block + 2 m_c
    square-sum columns).

Host side (O(N D) prep/reductions, exact in f64): fp8 casts, class
packing, ssq = sum_i ||hi_i+lo_i||^2, r2s = sum_i ||fp8 row_i||^2 ^2,
S2 from label counts, covering weights, and the final scalar assembly.

All input DMAs are issued from the SP sequencer in the exact order PE
consumes them (misc, rr, gsl chunks); per-chunk semaphores because HWDGE
queues complete out of order.  Semaphores are cleared inline by their
last waiter so the NEFF stays re-executable without a finalizer block.
"""

import numpy as np
import ml_dtypes

N, D, C = 4096, 1024, 1000
NCORES = 8
RROW = 512                # rows per core (balanced class packing)
KT = N // 128             # 32 gsl k-tiles
KT_R = RROW // 128        # 4 row k-tiles
PAD_LABEL = 999.0         # outside iota range [0,128) -> onehot row of zeros

# ordered covering design: each core computes the fixed block pattern
# {(0,0),(0,1),(0,2),(0,3),(1,2)} of its 4-slice bundle; quads are ordered so
# every slice is q0 exactly once (owns its diagonal) and all 28 pairs appear.
QUADS = [(0, 5, 4, 7), (3, 1, 0, 4), (2, 6, 0, 1), (4, 7, 6, 2),
         (7, 2, 3, 1), (5, 4, 7, 2), (1, 3, 5, 4), (6, 3, 5, 1)]
POS = [(0, 0), (0, 1), (0, 2), (0, 3), (1, 2)]  # device block order
NBLK = len(POS)

KTG = (N - RROW) // 128   # 28 gsl k-tiles (own 512 rows come via rr)
# gsl k-tile chunks: small tail chunks so the last DMA gates minimal PE work
GCH = [(0, 4), (4, 8), (8, 12), (12, 16), (16, 20), (20, 22), (22, 24),
       (24, 26), (26, 28)]

_CACHE = {}


def _build_module():
    import concourse.bass as bass
    import concourse.mybir as mybir
    from contextlib import ExitStack

    dt = mybir.dt
    AL = mybir.AluOpType
    AF = mybir.ActivationFunctionType
    DR = mybir.MatmulPerfMode.DoubleRow
    nc = bass.Bass("TRN2", target_bir_lowering=False, debug=False)

    gsl = nc.dram_tensor("gsl", [N - RROW, 512], dt.float8e4,
                         kind="ExternalInput").ap()
    rr = nc.dram_tensor("rr", [RROW, 2 * D], dt.float8e4, kind="ExternalInput").ap()
    misc = nc.dram_tensor("misc", [128, 256 + KT_R + 1], dt.float32,
                          kind="ExternalInput").ap()
    out1 = nc.dram_tensor("out1", [128, 2], dt.float32,
                          kind="ExternalOutput").ap()
    out2 = nc.dram_tensor("out2", [128, NBLK * 128], dt.bfloat16,
                          kind="ExternalOutput").ap()

    gsl_t = gsl.rearrange("(t p) d -> p t d", p=128)
    rr_t = rr.rearrange("(t p) d -> p t d", p=128)

    ctx = ExitStack()
    with ctx:
        sb = lambda shape, dtype, name: ctx.enter_context(
            nc.sbuf_tensor(name, shape, dtype)).ap()
        ps = lambda shape, name: ctx.enter_context(
            nc.psum_tensor(name, shape, dt.float32)).ap()

        gsl_sb = sb([128, KTG, 512], dt.float8e4, "gsl_sb")
        # per row tile: [hi half0 | hi half1 | lo half0 | lo half1]
        rr_sb = sb([128, KT_R, 2, 2, 512], dt.float8e4, "rr_sb")
        misc_sb = sb([128, 256 + KT_R + 1], dt.float32, "misc_sb")
        oh_sb = sb([128, KT_R, 2, 128], dt.float8e4, "oh_sb")
        scr = sb([128, 1, NBLK * 128], dt.bfloat16, "scr")  # squared G blocks
        scr_a = sb([128, 512], dt.bfloat16, "scr_a")    # throwaway ACT outs
        scr_b = sb([128, 512], dt.bfloat16, "scr_b")
        stats = sb([128, 2], dt.float32, "stats")

        pg0 = ps([128, 512], "pg0")      # blocks (0,0)..(0,3)
        pg12 = ps([128, 128], "pg12")    # block (1,2)
        pmh0 = ps([128, 512], "pmh0")    # m_c cols 0:512
        pmh1 = ps([128, 512], "pmh1")    # m_c cols 512:1024

        s_gs = [ctx.enter_context(nc.semaphore(f"s_gs{b}"))
                for b in range(len(GCH))]
        s_misc = ctx.enter_context(nc.semaphore("s_misc"))
        s_rr = [ctx.enter_context(nc.semaphore(f"s_rr{t}")) for t in range(2)]
        s_oh = ctx.enter_context(nc.semaphore("s_oh"))
        s_pe = ctx.enter_context(nc.semaphore("s_pe"))
        s_c = ctx.enter_context(nc.semaphore("s_c"))
        s_c0 = ctx.enter_context(nc.semaphore("s_c0"))
        s_o1 = ctx.enter_context(nc.semaphore("s_o1"))
        s_v = ctx.enter_context(nc.semaphore("s_v"))
        s_out = ctx.enter_context(nc.semaphore("s_out"))

        block_cm = nc.Block()
        block = block_cm.__enter__()

        # ---------------- SP: single ordered DMA queue ----------------------
        @block.sync
        def _(sync):
            sync.dma_start(misc_sb[:], misc).then_inc(s_misc, 16)
            sync.dma_start(rr_sb[:, 0:2], rr_t[:, 0:2, :]).then_inc(s_rr[0], 16)
            sync.dma_start(rr_sb[:, 2:4], rr_t[:, 2:4, :]).then_inc(s_rr[1], 16)
            for b, (k0, k1) in enumerate(GCH):
                sync.dma_start(gsl_sb[:, k0:k1, :],
                               gsl_t[:, k0:k1, :]).then_inc(s_gs[b], 16)
            sync.wait_ge(s_c, 1)     # ACT squared pg0 into scr
            sync.wait_ge(s_v, 1)     # DVE copied pg12 into scr
            sync.dma_start(out2, scr[:]).then_inc(s_out, 16)

        # ---------------- PE: M~ (hi+lo DoubleRow) then G blocks ------------
        @block.tensor
        def _(tensor):
            tensor.wait_ge(s_oh, 1)
            mm = None
            for t in range(KT_R):
                if t % 2 == 0:
                    tensor.wait_ge(s_rr[t // 2], 16)
                st, sp = (t == 0), (t == KT_R - 1)
                oh_t = oh_sb[:, t]
                nc.tensor.matmul(pmh0[:], oh_t, rr_sb[:, t, :, 0, :],
                                 start=st, stop=sp, perf_mode=DR)
                mm = nc.tensor.matmul(pmh1[:], oh_t, rr_sb[:, t, :, 1, :],
                                      start=st, stop=sp, perf_mode=DR)
            mm.then_inc(s_pe, 1)                                        # ->1 M~

            # G contribution of the core's own 512 rows, read from the hi
            # halves of rr (stored in quad column order): per k-tile pair,
            # 4 block matmuls into pg0 plus one into pg12
            def rrhi(t2, j):
                return rr_sb[:, t2:t2 + 2, 0, 0, 128 * j:128 * (j + 1)]

            for j2 in range(2):
                for j in range(4):
                    nc.tensor.matmul(pg0[:, 128 * j:128 * (j + 1)],
                                     rrhi(2 * j2, 0), rrhi(2 * j2, j),
                                     start=(j2 == 0 and j == 0), stop=False,
                                     perf_mode=DR)
                nc.tensor.matmul(pg12[:], rrhi(2 * j2, 1), rrhi(2 * j2, 2),
                                 start=(j2 == 0), stop=False, perf_mode=DR)

            for b, (k0, k1) in enumerate(GCH):
                tensor.wait_ge(s_gs[b], 16)
                for r in range(k0 // 2, k1 // 2):
                    sp = (r == KTG // 2 - 1)
                    lhs2 = gsl_sb[:, 2 * r:2 * r + 2, :]
                    nc.tensor.matmul(pg0[:], lhs2[:, :, 0:128],
                                     lhs2[:, :, 0:512],
                                     start=False, stop=sp, perf_mode=DR)
                    mm = nc.tensor.matmul(pg12[:], lhs2[:, :, 128:256],
                                          lhs2[:, :, 256:384],
                                          start=False, stop=sp, perf_mode=DR)
            mm.then_inc(s_pe, 1)                                        # ->2 G


        # ---------------- ACT: psum squares; early m_c stats DMA ------------
        @block.scalar
        def _(scalar):
            zbias = misc_sb[:, 256 + KT_R:256 + KT_R + 1]
            scalar.wait_ge(s_pe, 1)
            nc.scalar.activation(scr_a[:], pmh0[:], AF.Square, bias=zbias,
                                 accum_out=stats[:, 0:1])
            nc.scalar.activation(scr_b[:], pmh1[:], AF.Square, bias=zbias,
                                 accum_out=stats[:, 1:2]).then_inc(s_c0, 1)
            scalar.wait_ge(s_c0, 1)  # own engine drained (stats written)
            scalar.dma_start(out1, stats[:]).then_inc(s_o1, 16)

            scalar.wait_ge(s_pe, 2)
            nc.scalar.activation(scr[:, 0, 0:512], pg0[:], AF.Square,
                                 bias=zbias).then_inc(s_c, 1)
            scalar.wait_ge(s_o1, 16)

        # ---------------- DVE: onehots + 2 psum square-reduces --------------
        @block.vector
        def _(vector):
            vector.wait_ge(s_misc, 16)
            for t in range(KT_R):
                inst = nc.vector.tensor_scalar(
                    out=oh_sb[:, t], in0=misc_sb[:, 0:256],
                    scalar1=misc_sb[:, 256 + t:257 + t], scalar2=None,
                    op0=AL.is_equal,
                )
            inst.then_inc(s_oh, 1)

            vector.wait_ge(s_pe, 2)
            nc.vector.tensor_copy(scr[:, 0, 512:640],
                                  pg12[:]).then_inc(s_v, 1)


        block_cm.__exit__(None, None, None)

    # strip the framework's const-page memsets from the preamble: nothing
    # reads the const APs (Square bias comes from misc), and dropping them
    # releases the entry barrier ~400ns earlier
    pre = list(nc.m.functions[0].blocks)[0]
    pre.instructions = [i for i in pre.instructions
                        if type(i).__name__ != "InstMemset"]
    return nc


def _block_weights():
    W = np.zeros((NCORES, NBLK), dtype=np.float64)
    for a in range(8):
        for b in range(a, 8):
            for m, q in enumerate(QUADS):
                if a == b:
                    if q[0] == a:
                        W[m, 0] += 1.0
                        break
                elif a in q and b in q:
                    i, j = sorted((q.index(a), q.index(b)))
                    if (i, j) in POS:
                        W[m, POS.index((i, j))] += 2.0
                        break
            else:
                raise AssertionError(f"pair {(a, b)} uncovered")
    return W


def _pack_classes(t):
    """Greedy bin-pack classes into 8 cores: <=128 classes, <=RROW rows."""
    cnt = np.bincount(t, minlength=C)
    order = np.argsort(-cnt, kind="stable")
    bins = [[] for _ in range(NCORES)]
    loads = np.zeros(NCORES, dtype=np.int64)
    for c in order:
        for b in sorted(range(NCORES), key=lambda b: loads[b]):
            if len(bins[b]) < 128 and loads[b] + cnt[c] <= RROW:
                bins[b].append(int(c))
                loads[b] += cnt[c]
                break
        else:
            raise AssertionError("class packing failed; need padded fallback")
    return bins


def _prepare_inputs(output, target):
    A = np.ascontiguousarray(np.asarray(output, dtype=np.float32))
    t = np.asarray(target).astype(np.int64)
    F8 = A.astype(ml_dtypes.float8_e4m3)
    L8 = (A - F8.astype(np.float32)).astype(ml_dtypes.float8_e4m3)

    bins = _pack_classes(t)
    in_maps = []
    host = {}
    for m in range(NCORES):
        local = {c: i for i, c in enumerate(bins[m])}
        sel = np.nonzero(np.isin(t, bins[m]))[0]
        assert len(sel) <= RROW
        # permuted column order: the core's 4 quad slices first (so the
        # G-from-rr matmuls see them at fixed offsets), then the rest
        qcols = np.concatenate([np.arange(128 * q, 128 * (q + 1))
                                for q in QUADS[m]])
        pcols = np.concatenate(
            [qcols, np.setdiff1d(np.arange(D), qcols)])
        rr = np.zeros((RROW, 2 * D), dtype=ml_dtypes.float8_e4m3)
        lbl = np.full((RROW,), PAD_LABEL, dtype=np.float32)
        rr[:len(sel), 0:1024] = F8[sel][:, pcols]
        rr[:len(sel), 1024:2048] = L8[sel][:, pcols]
        lbl[:len(sel)] = np.array([local[int(c)] for c in t[sel]],
                                  dtype=np.float32)
        misc = np.zeros((128, 256 + KT_R + 1), dtype=np.float32)
        misc[:, 0:128] = np.arange(128, dtype=np.float32)[None, :]
        misc[:, 128:256] = np.arange(128, dtype=np.float32)[None, :]
        misc[:, 256:256 + KT_R] = lbl.reshape(KT_R, 128).T
        rest = np.setdiff1d(np.arange(N), sel)
        gsl = F8[np.ix_(rest, qcols)]
        in_maps.append({
            "gsl": np.ascontiguousarray(gsl),
            "rr": rr,
            "misc": misc,
        })

    # exact host-side reductions (f64) on the same fp8 data the device sees
    F = F8.astype(np.float64)
    L = L8.astype(np.float64)
    R = F + L
    host["ssq"] = float(np.einsum("ij,ij->", R, R))
    host["r2s"] = float((np.einsum("ij,ij->i", F, F) ** 2).sum())
    cnt = np.bincount(t, minlength=C).astype(np.float64)
    host["S2"] = ((cnt ** 2).sum() - N) / 2.0
    return in_maps, host


def _combine(partials, host):
    W = _block_weights()
    # partials: per core (out1 [128,2] f32 m_c halves, out2 [128,640] bf16
    # squared G blocks in POS order)
    P = np.stack([np.asarray(o2, dtype=np.float64).reshape(128, NBLK, 128)
                  for o1, o2 in partials])
    P[:, :, 4, :] **= 2  # block (1,2) is DMA'd raw; blocks 0..3 pre-squared
    gss = float((P.sum(axis=(1, 3)) * W).sum())
    msq = float(sum(np.asarray(o1, dtype=np.float64).sum()
                    for o1, o2 in partials))
    S3 = (gss - host["r2s"]) / 2.0
    S1 = (msq - host["ssq"]) / 2.0
    loss = -(S1 / (host["S2"] * np.sqrt(S3)))
    return np.float32(loss)


def kernel(output, target):
    from concourse.bass_utils import run_bass_kernel_spmd

    if "nc" not in _CACHE:
        _CACHE["nc"] = _build_module()
    nc = _CACHE["nc"]
    in_maps, host = _prepare_inputs(output, target)
    res = run_bass_kernel_spmd(nc, in_maps, core_ids=list(range(NCORES)))
    return _combine([(r["out1"], r["out2"]) for r in res.results], host)


# revision 26
# speedup vs baseline: 2.2336x; 1.1550x over previous
"""Trainium2 Bass kernel for nn_AlignmentLoss (8-core SPMD, no collectives).

Math: with gram = A A^T and eq[i,j] = (t_i == t_j), both symmetric,
  S1 = sum(tril(gram*eq,-1)) = (sum_c ||m_c||^2 - sum_i ||a_i||^2)/2
  S2 = sum(tril(eq,-1))      = (sum_c n_c^2 - N)/2
  S3 = sum(tril(gram,-1)^2)  = (||A^T A||_F^2 - sum_i (||a_i||^2)^2)/2
  loss = -(S1 / (S2 * sqrt(S3)))
where m_c = sum of rows with label c, n_c = count of label c.

Device work (the O(N D^2) + O(N D C/8) FLOPs):
  * S3 gram: G = F^T F with F = fp8e4(A).  G's 8x8 grid of 128x128 blocks
    is covered by giving each core 4 of the 8 column-slices (a covering
    design over slice pairs); each core computes the 5 blocks
    {(0,0),(0,1),(0,2),(0,3),(1,2)} of its 4-slice bundle and a
    host-chosen 0/1/2 weight per block makes every G block count exactly
    once (2x for off-diagonal).  Matmuls run in fp8 DoubleRow perf mode
    (two k-tiles per instruction).  Per-block square-sums via ACT
    Square+accum (3 blocks) and DVE tensor_tensor_reduce (2 blocks).
  * S1 class sums: rows are packed by class so each core holds <=128
    classes / exactly 512 rows; rows are stored as an fp8 hi/lo pair
    (hi = fp8(A), lo = fp8(A - hi)) and the onehot matmul uses DoubleRow
    with weights = (onehot, onehot), moving = (hi, lo), accumulating
    m_c = sum(hi+lo) at ~bf16 accuracy in one pass.  ||m_c||^2 partials
    via ACT Square+accum of the two psum banks.
  * Each core DMAs out a [128, 7] f32 stats tensor (5 G-block + 2 m_c
    square-sum columns).

Host side (O(N D) prep/reductions, exact in f64): fp8 casts, class
packing, ssq = sum_i ||hi_i+lo_i||^2, r2s = sum_i ||fp8 row_i||^2 ^2,
S2 from label counts, covering weights, and the final scalar assembly.

All input DMAs are issued from the SP sequencer in the exact order PE
consumes them (misc, rr, gsl chunks); per-chunk semaphores because HWDGE
queues complete out of order.  Semaphores are cleared inline by their
last waiter so the NEFF stays re-executable without a finalizer block.
"""

import numpy as np
import ml_dtypes

N, D, C = 4096, 1024, 1000
NCORES = 8
RROW = 512                # rows per core (balanced class packing)
KT = N // 128             # 32 gsl k-tiles
KT_R = RROW // 128        # 4 row k-tiles
PAD_LABEL = 999.0         # outside iota range [0,128) -> onehot row of zeros

# ordered covering design: each core computes the fixed block pattern
# {(0,0),(0,1),(0,2),(0,3),(1,2)} of its 4-slice bundle; quads are ordered so
# every slice is q0 exactly once (owns its diagonal) and all 28 pairs appear.
QUADS = [(0, 5, 4, 7), (3, 1, 0, 4), (2, 6, 0, 1), (4, 7, 6, 2),
         (7, 2, 3, 1), (5, 4, 7, 2), (1, 3, 5, 4), (6, 3, 5, 1)]
POS = [(0, 0), (0, 1), (0, 2), (0, 3), (1, 2)]  # device block order
NBLK = len(POS)

KTG = (N - RROW) // 128   # 28 gsl k-tiles (own 512 rows come via rr)
# gsl k-tile chunks, split between the SP and ACT sequencers so the shared
# HWDGE (625ns/DMA) paces the stream rather than one engine's SEQ rate
GCH = [(0, 4), (4, 8), (8, 12), (12, 16), (16, 20), (20, 24), (24, 26),
       (26, 28)]
SP_CH = [0, 2, 4, 6]      # chunk ids issued by SP (plus rr, misc)
ACT_CH = [1, 3, 5, 7]     # chunk ids issued by ACT

_CACHE = {}


def _build_module():
    import concourse.bass as bass
    import concourse.mybir as mybir
    from contextlib import ExitStack

    dt = mybir.dt
    AL = mybir.AluOpType
    AF = mybir.ActivationFunctionType
    DR = mybir.MatmulPerfMode.DoubleRow
    nc = bass.Bass("TRN2", target_bir_lowering=False, debug=False)

    gsl = nc.dram_tensor("gsl", [N - RROW, 512], dt.float8e4,
                         kind="ExternalInput").ap()
    rr = nc.dram_tensor("rr", [RROW, D], dt.float8e4,
                        kind="ExternalInput").ap()
    misc = nc.dram_tensor("misc", [128, 128 + KT_R + 1], dt.float32,
                          kind="ExternalInput").ap()
    out1 = nc.dram_tensor("out1", [128, 2], dt.float32,
                          kind="ExternalOutput").ap()
    out2 = nc.dram_tensor("out2", [128, NBLK * 128], dt.bfloat16,
                          kind="ExternalOutput").ap()

    gsl_t = gsl.rearrange("(t p) d -> p t d", p=128)
    rr_t = rr.rearrange("(t p) d -> p t d", p=128)

    ctx = ExitStack()
    with ctx:
        sb = lambda shape, dtype, name: ctx.enter_context(
            nc.sbuf_tensor(name, shape, dtype)).ap()
        ps = lambda shape, name: ctx.enter_context(
            nc.psum_tensor(name, shape, dt.float32)).ap()

        gsl_sb = sb([128, KTG, 512], dt.float8e4, "gsl_sb")
        # per row tile: [half0 | half1] of the fp8 hi row (permuted cols)
        rr_sb = sb([128, KT_R, 2, 512], dt.float8e4, "rr_sb")
        misc_sb = sb([128, 128 + KT_R + 1], dt.float32, "misc_sb")
        oh_sb = sb([128, KT_R, 128], dt.float8e4, "oh_sb")
        scr = sb([128, 1, NBLK * 128], dt.bfloat16, "scr")  # squared G blocks
        scr_a = sb([128, 512], dt.bfloat16, "scr_a")    # throwaway ACT outs
        scr_b = sb([128, 512], dt.bfloat16, "scr_b")
        stats = sb([128, 2], dt.float32, "stats")

        pg0 = ps([128, 512], "pg0")      # blocks (0,0)..(0,3)
        pg12 = ps([128, 128], "pg12")    # block (1,2)
        pmh0 = ps([128, 512], "pmh0")    # m_c cols 0:512
        pmh1 = ps([128, 512], "pmh1")    # m_c cols 512:1024

        s_gs = [ctx.enter_context(nc.semaphore(f"s_gs{b}"))
                for b in range(len(GCH))]
        s_misc = ctx.enter_context(nc.semaphore("s_misc"))
        s_rr = [ctx.enter_context(nc.semaphore("s_rr0"))]
        s_oh = ctx.enter_context(nc.semaphore("s_oh"))
        s_pe = ctx.enter_context(nc.semaphore("s_pe"))
        s_c = ctx.enter_context(nc.semaphore("s_c"))
        s_c0 = ctx.enter_context(nc.semaphore("s_c0"))
        s_o1 = ctx.enter_context(nc.semaphore("s_o1"))
        s_v = ctx.enter_context(nc.semaphore("s_v"))
        s_out = ctx.enter_context(nc.semaphore("s_out"))

        block_cm = nc.Block()
        block = block_cm.__enter__()

        # ---------------- SP + ACT: interleaved input DMA queues ------------
        @block.sync
        def _(sync):
            sync.dma_start(rr_sb[:], rr_t[:]).then_inc(s_rr[0], 16)
            sync.dma_start(misc_sb[:], misc).then_inc(s_misc, 16)
            for b in SP_CH:
                k0, k1 = GCH[b]
                sync.dma_start(gsl_sb[:, k0:k1, :],
                               gsl_t[:, k0:k1, :]).then_inc(s_gs[b], 16)
            sync.wait_ge(s_c, 1)     # ACT squared pg0 into scr
            sync.wait_ge(s_v, 1)     # DVE copied pg12 into scr
            sync.dma_start(out2, scr[:]).then_inc(s_out, 16)
            sync.wait_ge(s_out, 16)

        # ---------------- PE: M~ (hi+lo DoubleRow) then G blocks ------------
        @block.tensor
        def _(tensor):
            tensor.wait_ge(s_oh, 1)
            tensor.wait_ge(s_rr[0], 16)
            mm = None
            for j2 in range(KT_R // 2):
                st, sp = (j2 == 0), (j2 == KT_R // 2 - 1)
                oh2 = oh_sb[:, 2 * j2:2 * j2 + 2, :]
                nc.tensor.matmul(pmh0[:], oh2, rr_sb[:, 2 * j2:2 * j2 + 2, 0, :],
                                 start=st, stop=sp, perf_mode=DR)
                mm = nc.tensor.matmul(pmh1[:], oh2,
                                      rr_sb[:, 2 * j2:2 * j2 + 2, 1, :],
                                      start=st, stop=sp, perf_mode=DR)
            mm.then_inc(s_pe, 1)                                        # ->1 M~

            # G contribution of the core's own 512 rows, read from the hi
            # halves of rr (stored in quad column order): per k-tile pair,
            # 4 block matmuls into pg0 plus one into pg12
            def rrhi(t2, j):
                return rr_sb[:, t2:t2 + 2, 0, 128 * j:128 * (j + 1)]

            for j2 in range(2):
                for j in range(4):
                    nc.tensor.matmul(pg0[:, 128 * j:128 * (j + 1)],
                                     rrhi(2 * j2, 0), rrhi(2 * j2, j),
                                     start=(j2 == 0 and j == 0), stop=False,
                                     perf_mode=DR)
                nc.tensor.matmul(pg12[:], rrhi(2 * j2, 1), rrhi(2 * j2, 2),
                                 start=(j2 == 0), stop=False, perf_mode=DR)

            for b, (k0, k1) in enumerate(GCH):
                tensor.wait_ge(s_gs[b], 16)
                for r in range(k0 // 2, k1 // 2):
                    sp = (r == KTG // 2 - 1)
                    lhs2 = gsl_sb[:, 2 * r:2 * r + 2, :]
                    nc.tensor.matmul(pg0[:], lhs2[:, :, 0:128],
                                     lhs2[:, :, 0:512],
                                     start=False, stop=sp, perf_mode=DR)
                    mm = nc.tensor.matmul(pg12[:], lhs2[:, :, 128:256],
                                          lhs2[:, :, 256:384],
                                          start=False, stop=sp, perf_mode=DR)
            mm.then_inc(s_pe, 1)                                        # ->2 G


        # ---------------- ACT: psum squares; early m_c stats DMA ------------
        @block.scalar
        def _(scalar):
            scalar.wait_ge(s_misc, 16)   # let misc+rr win the first HWDGE slots
            for b in ACT_CH:
                k0, k1 = GCH[b]
                scalar.dma_start(gsl_sb[:, k0:k1, :],
                                 gsl_t[:, k0:k1, :]).then_inc(s_gs[b], 16)
            zbias = misc_sb[:, 128 + KT_R:128 + KT_R + 1]
            scalar.wait_ge(s_pe, 1)
            nc.scalar.activation(scr_a[:], pmh0[:], AF.Square, bias=zbias,
                                 accum_out=stats[:, 0:1])
            nc.scalar.activation(scr_b[:], pmh1[:], AF.Square, bias=zbias,
                                 accum_out=stats[:, 1:2]).then_inc(s_c0, 1)
            scalar.wait_ge(s_pe, 2)
            nc.scalar.activation(scr[:, 0, 0:512], pg0[:], AF.Square,
                                 bias=zbias).then_inc(s_c, 1)

        # ---------------- DVE: onehots + 2 psum square-reduces --------------
        @block.vector
        def _(vector):
            vector.wait_ge(s_misc, 16)
            for t in range(KT_R):
                inst = nc.vector.tensor_scalar(
                    out=oh_sb[:, t], in0=misc_sb[:, 0:128],
                    scalar1=misc_sb[:, 128 + t:129 + t], scalar2=None,
                    op0=AL.is_equal,
                )
            inst.then_inc(s_oh, 1)

            vector.wait_ge(s_pe, 2)
            nc.vector.tensor_copy(scr[:, 0, 512:640],
                                  pg12[:]).then_inc(s_v, 1)


        # ---------------- Pool: both outputs via SWDGE (HWDGE stays free) ---
        @block.gpsimd
        def _(g):
            g.wait_ge(s_c0, 1)       # ACT wrote stats cols
            g.dma_start(out1, stats[:]).then_inc(s_o1, 16)
            g.wait_ge(s_o1, 16)

        block_cm.__exit__(None, None, None)

    # Post-build surgery on the framework preamble/epilogue:
    #  * drop the const-page memsets — nothing reads the const APs (Square
    #    bias comes from a zeroed misc column);
    #  * drop the entry barrier (drains + sem butterfly) — every engine
    #    stream here is gated purely by data semaphores, and the preamble
    #    holds only per-engine register moves which order within each
    #    engine anyway;
    #  * drop the exit barrier sems (their wait thresholds assume the entry
    #    incs), keeping the per-engine exit drains.
    blks = list(nc.m.functions[0].blocks)
    pre, end = blks[0], blks[-1]
    pre.instructions = [
        i for i in pre.instructions
        if type(i).__name__ not in ("InstMemset", "InstDrain")
        and not str(getattr(i, "name", "")).startswith("barrier_")
    ]
    end.instructions = [
        i for i in end.instructions
        if not str(getattr(i, "name", "")).startswith("barrier_")
    ]
    return nc


def _block_weights():
    W = np.zeros((NCORES, NBLK), dtype=np.float64)
    for a in range(8):
        for b in range(a, 8):
            for m, q in enumerate(QUADS):
                if a == b:
                    if q[0] == a:
                        W[m, 0] += 1.0
                        break
                elif a in q and b in q:
                    i, j = sorted((q.index(a), q.index(b)))
                    if (i, j) in POS:
                        W[m, POS.index((i, j))] += 2.0
                        break
            else:
                raise AssertionError(f"pair {(a, b)} uncovered")
    return W


def _pack_classes(t):
    """Greedy bin-pack classes into 8 cores: <=128 classes, <=RROW rows."""
    cnt = np.bincount(t, minlength=C)
    order = np.argsort(-cnt, kind="stable")
    bins = [[] for _ in range(NCORES)]
    loads = np.zeros(NCORES, dtype=np.int64)
    for c in order:
        for b in sorted(range(NCORES), key=lambda b: loads[b]):
            if len(bins[b]) < 128 and loads[b] + cnt[c] <= RROW:
                bins[b].append(int(c))
                loads[b] += cnt[c]
                break
        else:
            raise AssertionError("class packing failed; need padded fallback")
    return bins


def _prepare_inputs(output, target):
    A = np.ascontiguousarray(np.asarray(output, dtype=np.float32))
    t = np.asarray(target).astype(np.int64)
    F8 = A.astype(ml_dtypes.float8_e4m3)

    bins = _pack_classes(t)
    in_maps = []
    host = {}
    for m in range(NCORES):
        local = {c: i for i, c in enumerate(bins[m])}
        sel = np.nonzero(np.isin(t, bins[m]))[0]
        assert len(sel) <= RROW
        # permuted column order: the core's 4 quad slices first (so the
        # G-from-rr matmuls see them at fixed offsets), then the rest
        qcols = np.concatenate([np.arange(128 * q, 128 * (q + 1))
                                for q in QUADS[m]])
        pcols = np.concatenate(
            [qcols, np.setdiff1d(np.arange(D), qcols)])
        rr = np.zeros((RROW, D), dtype=ml_dtypes.float8_e4m3)
        lbl = np.full((RROW,), PAD_LABEL, dtype=np.float32)
        rr[:len(sel)] = F8[sel][:, pcols]
        lbl[:len(sel)] = np.array([local[int(c)] for c in t[sel]],
                                  dtype=np.float32)
        misc = np.zeros((128, 128 + KT_R + 1), dtype=np.float32)
        misc[:, 0:128] = np.arange(128, dtype=np.float32)[None, :]
        misc[:, 128:128 + KT_R] = lbl.reshape(KT_R, 128).T
        rest = np.setdiff1d(np.arange(N), sel)
        gsl = F8[np.ix_(rest, qcols)]
        in_maps.append({
            "gsl": np.ascontiguousarray(gsl),
            "rr": rr,
            "misc": misc,
        })

    # exact host-side reductions (f64) on the same fp8 data the device sees
    F = F8.astype(np.float64)
    host["ssq"] = float(np.einsum("ij,ij->", F, F))
    host["r2s"] = float((np.einsum("ij,ij->i", F, F) ** 2).sum())
    cnt = np.bincount(t, minlength=C).astype(np.float64)
    host["S2"] = ((cnt ** 2).sum() - N) / 2.0
    return in_maps, host


def _combine(partials, host):
    W = _block_weights()
    # partials: per core (out1 [128,2] f32 m_c halves, out2 [128,640] bf16
    # squared G blocks in POS order)
    P = np.stack([np.asarray(o2, dtype=np.float64).reshape(128, NBLK, 128)
                  for o1, o2 in partials])
    P[:, :, 4, :] **= 2  # block (1,2) is DMA'd raw; blocks 0..3 pre-squared
    gss = float((P.sum(axis=(1, 3)) * W).sum())
    msq = float(sum(np.asarray(o1, dtype=np.float64).sum()
                    for o1, o2 in partials))
    S3 = (gss - host["r2s"]) / 2.0
    S1 = (msq - host["ssq"]) / 2.0
    loss = -(S1 / (host["S2"] * np.sqrt(S3)))
    return np.float32(loss)


def kernel(output, target):
    from concourse.bass_utils import run_bass_kernel_spmd

    if "nc" not in _CACHE:
        _CACHE["nc"] = _build_module()
    nc = _CACHE["nc"]
    in_maps, host = _prepare_inputs(output, target)
    res = run_bass_kernel_spmd(nc, in_maps, core_ids=list(range(NCORES)))
    return _combine([(r["out1"], r["out2"]) for r in res.results], host)
